# revision 2
# baseline (speedup 1.0000x reference)
"""Trainium2 Bass kernel for spatial attention (nn_Attention_11407433138897).

Reference computation (B=16, C=512, H=W=32, 4 heads x 128 dim_head):
  qkv = 1x1conv(fmap)                      # [b, 3*512, n],  n = 1024
  sim = (q*scale) @ k^T + (q*scale) @ emb^T
  out = softmax(sim) @ v                   # -> [b, 512, 32, 32]

Key algebraic fold: sim = qs @ (k + emb)^T  -- the positional-bias matmul is
folded into k.  Softmax is computed without max-subtraction (logits ~N(0,1);
exp is safe in fp32/bf16 range).

Distribution: pure data-parallel over batch, 2 batches per NeuronCore, no
collectives.  Matmuls run in bf16 (fp32 PSUM accumulation); q-scale folded
into the weight on the host.

Per-core engine budget (v3): PE ~101us of matmul streaming is the roofline;
ACT ~72us of exp; DVE was ~90us in v2 (nearly PE-load!) and caused steady
~0.8us PSUM-slot stalls every few jc steps.  v3 changes:
  * Softmax padd tree split DVE/GpSimd: the 3 early adds of each head's
    exp-chunk reduction tree run on the otherwise-idle GpSimd engine
    (2.4us/op there vs 0.7 on DVE, but off the critical DVE queue), cutting
    DVE to ~65us so PSUM consumers (q copy/k add/v cast) run promptly.
  * GpSimd ucode warm-up add in the DMA lead-in (first tensor op on GpSimd
    otherwise pays a ~6us IRAM load mid-attention).
  * DMA schedule: c-pipelined chunk order so the first v-group's matmuls
    start as soon as (wtv[c], x[c]) land; critical loads on the two HW
    queues + gpsimd SWDGE; x[b1] moved to sync's tail so GpSimd is free
    for padds during attention.
  * v0 g2/g3 moved from the pre-attention phase into h0's filler slots
    (attention starts as soon as k0/q0 land; more filler supply).
  * Explicit per-jc filler schedule (next head's k/q early in each head,
    v groups later) instead of Bresenham pacing.
  * Last-head normalize quarter-split (256-wide recip/mul/DMA pipelined
    across sync+scalar queues) to shorten the serial tail.
"""

import os
import sys

import numpy as np
import ml_dtypes

sys.path.insert(0, "/opt/trn_rl_repo")
sys.path.insert(0, "/root/.axon_site")
sys.path.insert(0, "/root/.axon_site/_ro/trn_rl_repo")
sys.path.insert(0, "/root/.axon_site/_ro/pypackages")

HEADS = 4
D = 128           # dim_head
DIM = 512         # input channels
N = 1024          # 32*32 spatial positions
B = 16
N_CORES = 8
B_PER_CORE = B // N_CORES   # 2
SCALE = D ** -0.5
NH = 512          # half of n (PSUM bank = 512 fp32)
NJ = N // 128     # 8 j-chunks
CC = DIM // 128   # 4 contraction chunks

_BF16 = ml_dtypes.bfloat16

_COMPILED = {}


def _patch_tail_barrier(tile):
    """Slim TileContext epilogue: keep the sync drain (DMA-queue flush gated
    on the global semaphore clock = output integrity), drop the per-engine
    drains, semaphore clears, and second barrier (~4-6us of fixed tail for a
    single top-level context)."""
    from concourse.tile import ScopedClock

    def _drain_and_barrier(self, tick_clock, wait_clock):
        drain_inst = self.nc.sync.drain()
        wait_clock.add_sem_waits(
            drain_inst.ins, ScopedClock({None: tick_clock.global_clock})
        )
        self.nc.all_engine_barrier(sem_only=True)
        popped = self.nc._tile_sem_poison_stack.pop()
        assert popped is self._sem_poison

    tile.TileContext._drain_and_barrier = _drain_and_barrier


def _build():
    """Build + compile the per-core Bass graph (cached)."""
    import concourse.bass as bass
    import concourse.tile as tile
    from concourse import bacc, mybir

    if os.environ.get("KERNEL_SLIM_TAIL", "1") == "1":
        _patch_tail_barrier(tile)

    bf16 = mybir.dt.bfloat16
    f32 = mybir.dt.float32
    AF = mybir.ActivationFunctionType

    nc = bacc.Bacc("TRN2", target_bir_lowering=False, debug=False,
                   num_devices=N_CORES)

    x_dram = nc.dram_tensor("x", [B_PER_CORE, DIM, N], bf16, kind="ExternalInput")
    wt_dram = nc.dram_tensor("wt", [DIM, 3 * DIM], bf16, kind="ExternalInput")
    embt_dram = nc.dram_tensor("embt", [D, N], bf16, kind="ExternalInput")
    out_dram = nc.dram_tensor("out", [B_PER_CORE, HEADS * D, N], bf16,
                              kind="ExternalOutput")

    with tile.TileContext(nc) as tc:
        with (
            tc.tile_pool(name="const", bufs=1) as const_pool,
            tc.tile_pool(name="xin", bufs=1) as x_pool,
            tc.tile_pool(name="qkv", bufs=1) as qkv_pool,
            tc.tile_pool(name="expsim", bufs=6) as exp_pool,
            tc.tile_pool(name="padd", bufs=4) as padd_pool,
            tc.tile_pool(name="rec", bufs=3) as rec_pool,
            tc.tile_pool(name="outsb", bufs=3) as out_pool,
            tc.tile_pool(name="wide_ps", bufs=2, space="PSUM") as wide_ps,
            tc.tile_pool(name="pv_ps", bufs=2, space="PSUM") as pv_ps,
        ):
            # ---- input DMAs, c-pipelined so group matmuls can chase chunk
            # arrivals.  HW queues: sync = x0/nh0 -> wtk -> x1; scalar =
            # wtv -> wtq.  SWDGE (gpsimd) = x0/nh1 -> embt, then gpsimd is
            # free for attention-phase padds. ----
            x_sb = [[[x_pool.tile([128, NH], bf16, tag=f"x{b}_{c}_{nh}",
                                  name=f"x{b}_{c}_{nh}")
                      for nh in range(2)] for c in range(CC)]
                    for b in range(B_PER_CORE)]
            wtv_sb = [const_pool.tile([128, DIM], bf16, tag=f"wtv{c}",
                                      name=f"wtv{c}") for c in range(CC)]
            wtq_sb = [const_pool.tile([128, DIM], bf16, tag=f"wtq{c}",
                                      name=f"wtq{c}") for c in range(CC)]
            wtk_sb = [const_pool.tile([128, DIM], bf16, tag=f"wtk{c}",
                                      name=f"wtk{c}") for c in range(CC)]
            embt_sb = const_pool.tile([D, N], bf16, tag="embt")

            for c in range(CC):
                nc.sync.dma_start(x_sb[0][c][0][:],
                                  x_dram[0, bass.ts(c, 128), 0:NH])
            for c in range(CC):
                nc.scalar.dma_start(wtv_sb[c][:],
                                    wt_dram[bass.ts(c, 128), 2 * DIM:3 * DIM])
            for c in range(CC):
                nc.gpsimd.dma_start(x_sb[0][c][1][:],
                                    x_dram[0, bass.ts(c, 128), NH:N])
            nc.gpsimd.dma_start(embt_sb[:], embt_dram[:])
            for c in range(CC):
                nc.sync.dma_start(wtk_sb[c][:],
                                  wt_dram[bass.ts(c, 128), DIM:2 * DIM])
            for c in range(CC):
                nc.scalar.dma_start(wtq_sb[c][:], wt_dram[bass.ts(c, 128), 0:DIM])
            for c in range(CC):
                for nh in range(2):
                    nc.sync.dma_start(x_sb[1][c][nh][:],
                                      x_dram[1, bass.ts(c, 128),
                                             bass.ts(nh, NH)])

            # ---- constants (memsets on vector so the PE warm-up isn't
            # stuck behind DMA-issue occupancy) ----
            warm_sb = const_pool.tile([128, NH], bf16, tag="warm")
            nc.vector.memset(warm_sb[:], 1.0)
            ones128 = const_pool.tile([128, 128], bf16, tag="ones128")
            nc.vector.memset(ones128[:], 1.0)

            # preload the exp table set on ACT during the DMA wait (~2.7us
            # one-time ACT_TABLE_LOAD would otherwise land on the first
            # real exp mid-kernel)
            exp_warm = const_pool.tile([1, 8], bf16, tag="exp_warm")
            nc.scalar.activation(exp_warm[:], warm_sb[0:1, 0:8], AF.Exp)

            # GpSimd ucode warm-up: the first tensor op on GpSimd pays a
            # ~6us IRAM load; pay it here in the DMA lead-in, not at the
            # first padd mid-attention.
            gp_warm = const_pool.tile([128, 8], bf16, tag="gp_warm")
            nc.gpsimd.tensor_add(gp_warm[:], warm_sb[:, 0:8], warm_sb[:, 0:8])

            # ---- PE warm-up: junk matmuls while input DMAs are in flight;
            # flips the HAM clock gate toward 2.4 GHz before real work ----
            warm_ps = wide_ps.tile([128, 2 * NH], f32, tag="w", name="warm_ps")
            for i in range(10):
                nc.tensor.matmul(warm_ps[:, bass.ts(i % 2, NH)],
                                 warm_sb[:, 0:128], warm_sb[:],
                                 start=True, stop=True)
            warm_out = const_pool.tile([1, 8], f32, tag="warm_out")
            nc.vector.tensor_copy(warm_out[:], warm_ps[0:1, 0:8])
            warm_dram = nc.dram_tensor("warm_scratch", [1, 8], f32)
            nc.scalar.dma_start(warm_dram[:], warm_out[:])

            # ---- qkv staging (per-batch tags; no WAR serialization) ----
            q_sb = [qkv_pool.tile([128, HEADS * N], bf16, tag=f"q{b}",
                                  name=f"q{b}") for b in range(B_PER_CORE)]
            k_sb = [qkv_pool.tile([128, HEADS * N], bf16, tag=f"k{b}",
                                  name=f"k{b}") for b in range(B_PER_CORE)]
            v_sb = [qkv_pool.tile([128, NJ * DIM], bf16, tag=f"v{b}",
                                  name=f"v{b}") for b in range(B_PER_CORE)]

            # ---- projection group emitters (each: one wide PSUM tile,
            # 8 accumulating matmuls, one wide DVE consumer) ----
            def emit_qk_group(b, h, which):
                ps = wide_ps.tile([128, 2 * NH], f32, tag="w",
                                  name=f"{which}{b}_{h}")
                wt_t = wtq_sb if which == "q" else wtk_sb
                for c in range(CC):
                    for nh in range(2):
                        nc.tensor.matmul(
                            ps[:, bass.ts(nh, NH)],
                            wt_t[c][:, bass.ts(h, 128)],
                            x_sb[b][c][nh][:],
                            start=(c == 0), stop=(c == CC - 1),
                        )
                if which == "q":
                    nc.vector.tensor_copy(q_sb[b][:, h * N:(h + 1) * N], ps[:])
                else:
                    nc.vector.tensor_add(k_sb[b][:, h * N:(h + 1) * N],
                                         ps[:], embt_sb[:])

            def emit_v_group(b, g):
                # covers j-chunks 2g, 2g+1 -> v_sb cols [g*1024, (g+1)*1024)
                ps = wide_ps.tile([128, 2 * NH], f32, tag="w", name=f"v{b}_{g}")
                for c in range(CC):
                    for jo in range(2):
                        j = 2 * g + jo
                        nc.tensor.matmul(
                            ps[:, bass.ts(jo, NH)],
                            x_sb[b][c][j // 4][:, bass.ts(j % 4, 128)],
                            wtv_sb[c][:],
                            start=(c == 0), stop=(c == CC - 1),
                        )
                nc.vector.tensor_copy(v_sb[b][:, bass.ts(g, 2 * NH)], ps[:])

            # ---- attention for one head.  fillers: {jc: [closures]} with
            # the next head's k/q emitted early in each head and v groups
            # later.  Softmax denominators: pairwise exp sums reduced by an
            # adder tree; the three EARLY tree adds (pa01, pa23, their sum)
            # run on GpSimd, the three LATE ones (pa45, pa67, final) on DVE
            # (fast, off the deferred-finish critical path only by 3 jc).
            # The ones-matmul pair + recip/mul/DMA is DEFERRED into the next
            # head's jc-loop.  The last head instead accumulates exp chunks
            # directly (short tail, and the sums matmuls double as filler
            # for its otherwise-empty PE slots). ----
            def emit_attn_head(b, h, fillers, last, deferred):
                q_h = q_sb[b][:, h * N:(h + 1) * N]
                k_h = k_sb[b][:, h * N:(h + 1) * N]
                pv = pv_ps.tile([128, 2 * NH], f32, tag="pv",
                                name=f"pv{b}_{h}")
                sums_w = None
                exs = [None] * NJ
                padds = {}

                for jc in range(NJ):
                    if deferred is not None and jc == (0 if last else 3):
                        deferred()
                        deferred = None
                    sim = wide_ps.tile([128, 2 * NH], f32, tag="w",
                                       name=f"sim{b}_{h}_{jc}")
                    for ih in range(2):
                        nc.tensor.matmul(
                            sim[:, bass.ts(ih, NH)],
                            k_h[:, bass.ts(jc, 128)],
                            q_h[:, bass.ts(ih, NH)],
                            start=True, stop=True,
                        )
                    ex = exp_pool.tile([128, 2 * NH], bf16, tag="exp",
                                       name=f"ex{b}_{h}_{jc}")
                    if last and jc == NJ - 1:
                        # split final exp: the ih0 tail chain unblocks after
                        # 720ns instead of 1113ns
                        for ih in range(2):
                            nc.scalar.activation(ex[:, bass.ts(ih, NH)],
                                                 sim[:, bass.ts(ih, NH)],
                                                 AF.Exp)
                    else:
                        nc.scalar.activation(ex[:], sim[:], AF.Exp)
                    exs[jc] = ex
                    # pv lags one jc so its LDWEIGHTS prefetches during the
                    # sim stream instead of serializing after the sem wait
                    if jc > 0:
                        for ih in range(2):
                            nc.tensor.matmul(
                                pv[:, bass.ts(ih, NH)],
                                v_sb[b][:, (jc - 1) * NH + h * 128:
                                        (jc - 1) * NH + h * 128 + 128],
                                exs[jc - 1][:, bass.ts(ih, NH)],
                                start=(jc == 1), stop=False,
                            )
                    if last:
                        # semi-direct: padd for ex0+ex1, then direct pairs
                        if jc == 1:
                            pa = padd_pool.tile([128, 2 * NH], bf16,
                                                tag="padd",
                                                name=f"pa{b}_{h}_01")
                            nc.vector.tensor_add(pa[:], exs[0][:], exs[1][:])
                            padds["01"] = pa
                        elif jc == 2:
                            sums_w = pv_ps.tile([128, 2 * NH], f32, tag="pv",
                                                name=f"sums{b}_{h}")
                            for ih in range(2):
                                nc.tensor.matmul(
                                    sums_w[:, bass.ts(ih, NH)], ones128[:],
                                    padds["01"][:, bass.ts(ih, NH)],
                                    start=True, stop=False,
                                )
                        elif jc >= 3:
                            for ih in range(2):
                                nc.tensor.matmul(
                                    sums_w[:, bass.ts(ih, NH)], ones128[:],
                                    exs[jc - 1][:, bass.ts(ih, NH)],
                                    start=False, stop=False,
                                )
                    else:
                        if jc == 1:
                            pa = padd_pool.tile([128, 2 * NH], bf16,
                                                tag="padd_g0",
                                                name=f"pa{b}_{h}_01")
                            nc.gpsimd.tensor_add(pa[:], exs[0][:], exs[1][:])
                            padds["01"] = pa
                        elif jc == 3:
                            pa = padd_pool.tile([128, 2 * NH], bf16,
                                                tag="padd_g1",
                                                name=f"pa{b}_{h}_23")
                            nc.gpsimd.tensor_add(pa[:], exs[2][:], exs[3][:])
                            padds["23"] = pa
                        elif jc == 5:
                            pa = padd_pool.tile([128, 2 * NH], bf16,
                                                tag="padd_g2",
                                                name=f"pa2{b}_{h}_a")
                            nc.gpsimd.tensor_add(pa[:], padds["01"][:],
                                                 padds["23"][:])
                            padds["0123"] = pa
                            pa = padd_pool.tile([128, 2 * NH], bf16,
                                                tag="padd",
                                                name=f"pa{b}_{h}_45")
                            nc.vector.tensor_add(pa[:], exs[4][:], exs[5][:])
                            padds["45"] = pa
                        elif jc == 7:
                            pa = padd_pool.tile([128, 2 * NH], bf16,
                                                tag="padd2",
                                                name=f"pa{b}_{h}_67")
                            nc.vector.tensor_add(pa[:], exs[6][:], exs[7][:])
                            padds["67"] = pa
                    for f in fillers.get(jc, ()):
                        f()

                if not last:
                    # final pv pair (j-chunk NJ-1)
                    for ih in range(2):
                        nc.tensor.matmul(
                            pv[:, bass.ts(ih, NH)],
                            v_sb[b][:, (NJ - 1) * NH + h * 128:
                                    (NJ - 1) * NH + h * 128 + 128],
                            exs[NJ - 1][:, bass.ts(ih, NH)],
                            start=False, stop=True,
                        )
                if last:
                    # final direct sum contribution + immediate normalize,
                    # quarter-split (256-wide) recip/mul/DMA pipelined
                    # across sync/scalar DMA queues
                    for ih in range(2):
                        nc.tensor.matmul(
                            sums_w[:, bass.ts(ih, NH)], ones128[:],
                            exs[NJ - 1][:, bass.ts(ih, NH)],
                            start=False, stop=True,
                        )
                        nc.tensor.matmul(
                            pv[:, bass.ts(ih, NH)],
                            v_sb[b][:, (NJ - 1) * NH + h * 128:
                                    (NJ - 1) * NH + h * 128 + 128],
                            exs[NJ - 1][:, bass.ts(ih, NH)],
                            start=False, stop=True,
                        )
                    rec = rec_pool.tile([128, 2 * NH], f32, tag="rec",
                                        name=f"rec{b}_{h}")
                    o = out_pool.tile([128, 2 * NH], bf16, tag="o",
                                      name=f"o{b}_{h}")
                    NQ = NH // 2
                    for iq in range(4):
                        nc.vector.reciprocal_approx_fast(
                            out=rec[:, bass.ts(iq, NQ)],
                            in_=sums_w[:, bass.ts(iq, NQ)])
                        nc.vector.tensor_mul(o[:, bass.ts(iq, NQ)],
                                             pv[:, bass.ts(iq, NQ)],
                                             rec[:, bass.ts(iq, NQ)])
                        eng = nc.sync if iq % 2 == 0 else nc.scalar
                        eng.dma_start(
                            out_dram[b, h * D:(h + 1) * D, bass.ts(iq, NQ)],
                            o[:, bass.ts(iq, NQ)])
                    return None

                # late tree levels (consumed by the deferred finisher)
                pa2_1 = padd_pool.tile([128, 2 * NH], bf16, tag="padd2",
                                       name=f"pa2{b}_{h}_b")
                nc.vector.tensor_add(pa2_1[:], padds["45"][:], padds["67"][:])
                pa3 = padd_pool.tile([128, 2 * NH], bf16, tag="padd3",
                                     name=f"pa3{b}_{h}")
                nc.vector.tensor_add(pa3[:], padds["0123"][:], pa2_1[:])

                def finish():
                    sums = wide_ps.tile([128, 2 * NH], f32, tag="w",
                                        name=f"sums{b}_{h}")
                    for ih in range(2):
                        nc.tensor.matmul(
                            sums[:, bass.ts(ih, NH)], ones128[:],
                            pa3[:, bass.ts(ih, NH)],
                            start=True, stop=True,
                        )
                    rec = rec_pool.tile([128, 2 * NH], f32, tag="rec",
                                        name=f"rec{b}_{h}")
                    nc.vector.reciprocal_approx_fast(out=rec[:], in_=sums[:])
                    o = out_pool.tile([128, 2 * NH], bf16, tag="o",
                                      name=f"o{b}_{h}")
                    nc.vector.tensor_mul(o[:], pv[:], rec[:])
                    nc.sync.dma_start(out_dram[b, h * D:(h + 1) * D, :], o[:])

                return finish

            # ---- program order ----
            # pre-attention: batch-0 prerequisites, DMA-paced
            emit_v_group(0, 0)
            emit_v_group(0, 1)
            emit_v_group(0, 2)
            emit_v_group(0, 3)
            emit_qk_group(0, 0, "k")
            emit_qk_group(0, 0, "q")

            # per-head filler schedules: next head's k/q early (their DVE
            # consumers finish before that head starts), v groups later
            FILL = {
                (0, 0): {1: [lambda: emit_qk_group(0, 1, "k")],
                         4: [lambda: emit_qk_group(0, 1, "q")]},
                (0, 1): {1: [lambda: emit_qk_group(0, 2, "k")],
                         4: [lambda: emit_qk_group(0, 2, "q")],
                         6: [lambda: emit_v_group(1, 0)]},
                (0, 2): {1: [lambda: emit_qk_group(0, 3, "k")],
                         4: [lambda: emit_qk_group(0, 3, "q")],
                         6: [lambda: emit_v_group(1, 1)]},
                (0, 3): {1: [lambda: emit_qk_group(1, 0, "k")],
                         3: [lambda: emit_qk_group(1, 0, "q")],
                         5: [lambda: emit_v_group(1, 2)],
                         7: [lambda: emit_v_group(1, 3)]},
                (1, 0): {1: [lambda: emit_qk_group(1, 1, "k")],
                         4: [lambda: emit_qk_group(1, 1, "q")]},
                (1, 1): {1: [lambda: emit_qk_group(1, 2, "k")],
                         4: [lambda: emit_qk_group(1, 2, "q")]},
                (1, 2): {1: [lambda: emit_qk_group(1, 3, "k")],
                         4: [lambda: emit_qk_group(1, 3, "q")]},
                (1, 3): {},
            }

            deferred = None
            for b in range(B_PER_CORE):
                for h in range(HEADS):
                    deferred = emit_attn_head(
                        b, h, FILL[(b, h)],
                        last=(b == B_PER_CORE - 1 and h == HEADS - 1),
                        deferred=deferred)

    nc.compile()
    return nc


def _get_compiled():
    if "nc" not in _COMPILED:
        _COMPILED["nc"] = _build()
    return _COMPILED["nc"]


def _run(fmap, w_qkv, emb_h, emb_w, **spmd_kwargs):
    from concourse.bass_utils import run_bass_kernel_spmd

    nc = _get_compiled()

    fmap = np.asarray(fmap, dtype=np.float32)
    w_qkv = np.asarray(w_qkv, dtype=np.float32)
    emb_h = np.asarray(emb_h, dtype=np.float32)
    emb_w = np.asarray(emb_w, dtype=np.float32)

    b, c, hh, ww = fmap.shape
    x = fmap.reshape(b, c, hh * ww)

    # fold q scale into weight rows, transpose to [c, o], cast to bf16
    w = w_qkv.copy()
    w[:HEADS * D] *= SCALE
    wt = np.ascontiguousarray(w.T).astype(_BF16)

    embt = np.ascontiguousarray(
        (emb_h[:, None, :] + emb_w[None, :, :]).reshape(N, D).T
    ).astype(_BF16)

    x16 = x.astype(_BF16)
    in_maps = [
        {
            "x": np.ascontiguousarray(x16[i * B_PER_CORE:(i + 1) * B_PER_CORE]),
            "wt": wt,
            "embt": embt,
        }
        for i in range(N_CORES)
    ]

    res = run_bass_kernel_spmd(nc, in_maps, core_ids=list(range(N_CORES)),
                               **spmd_kwargs)
    out = np.concatenate(
        [np.asarray(res.results[i]["out"], dtype=np.float32)
         for i in range(N_CORES)], axis=0)
    return out.reshape(B, HEADS * D, hh, ww), res


def kernel(fmap, w_qkv, emb_h, emb_w):
    out, _ = _run(fmap, w_qkv, emb_h, emb_w)
    return out


if __name__ == "__main__":
    rng = np.random.default_rng(0)
    fmap = rng.standard_normal((B, DIM, 32, 32), dtype=np.float32)
    w_qkv = rng.standard_normal((3 * HEADS * D, DIM), dtype=np.float32) * DIM ** -0.5
    emb_h = rng.standard_normal((32, D), dtype=np.float32) * SCALE
    emb_w = rng.standard_normal((32, D), dtype=np.float32) * SCALE
    out = kernel(fmap=fmap, w_qkv=w_qkv, emb_h=emb_h, emb_w=emb_w)
    print("kernel out:", out.shape, out.dtype)


# revision 3
# speedup vs baseline: 1.1299x; 1.1299x over previous
"""Trainium2 Bass kernel for spatial attention (nn_Attention_11407433138897).

Reference computation (B=16, C=512, H=W=32, 4 heads x 128 dim_head):
  qkv = 1x1conv(fmap)                      # [b, 3*512, n],  n = 1024
  sim = (q*scale) @ k^T + (q*scale) @ emb^T
  out = softmax(sim) @ v                   # -> [b, 512, 32, 32]

Key algebraic fold: sim = qs @ (k + emb)^T  -- the positional-bias matmul is
folded into k.  Softmax is computed without max-subtraction (logits ~N(0,1)).

Distribution: pure data-parallel over batch, 2 batches per NeuronCore, no
collectives.  Matmuls in bf16 (fp32 PSUM accumulation); q-scale folded into
the weight on the host.

Per-core engine budget: PE ~101us of matmul streaming is the roofline; ACT
~72us of exp; DVE ~90us in v2 caused steady PSUM-consumer stalls.  v4:
  * Softmax padd tree split DVE/GpSimd on filler-rich heads; on the
    filler-poor back heads (b1 h1..h3) the denominators are instead
    accumulated by direct ones-matmuls (start/stop groups interleaved with
    sim/pv) -- the PE is starved there anyway, so the sums ride for free
    and those heads need no tree at all.
  * GpSimd ucode warm-up add in the DMA lead-in (first GpSimd tensor op
    otherwise pays a ~6us IRAM load mid-attention).
  * DMA schedule: c-pipelined chunk order on sync(x0/nh0, wtk, x1),
    scalar(wtv, wtq), gpsimd-SWDGE(x0/nh1, embt); GpSimd then free for
    attention-phase padds.
  * Fillers emitted as HALF-groups (4 matmuls) spread over the jc steps of
    each head (next head's k/q early; b1 v groups later), so nearly every
    jc has PE work beyond sim+pv and the sim[jc+1]-waits-exp[jc] PSUM-slot
    serialization stays hidden.
  * Deferred normalize (ones-matmul pair + recip + mul + DMA) emitted at
    jc3 of the next head for tree heads, jc0 for direct-sums heads (their
    sums tile needs the pv-pool slot by jc2).
  * Last-head normalize quarter-split (256-wide recip/mul/DMA pipelined
    across sync+scalar queues) to shorten the serial tail.
"""

import os
import sys

import numpy as np
import ml_dtypes

sys.path.insert(0, "/opt/trn_rl_repo")
sys.path.insert(0, "/root/.axon_site")
sys.path.insert(0, "/root/.axon_site/_ro/trn_rl_repo")
sys.path.insert(0, "/root/.axon_site/_ro/pypackages")

HEADS = 4
D = 128           # dim_head
DIM = 512         # input channels
N = 1024          # 32*32 spatial positions
B = 16
N_CORES = 8
B_PER_CORE = B // N_CORES   # 2
SCALE = D ** -0.5
NH = 512          # half of n (PSUM bank = 512 fp32)
NJ = N // 128     # 8 j-chunks
CC = DIM // 128   # 4 contraction chunks

_BF16 = ml_dtypes.bfloat16

_COMPILED = {}


def _patch_tail_barrier(tile):
    """Slim TileContext epilogue: keep the sync drain (DMA-queue flush gated
    on the global semaphore clock = output integrity), drop the per-engine
    drains, semaphore clears, and second barrier."""
    from concourse.tile import ScopedClock

    def _drain_and_barrier(self, tick_clock, wait_clock):
        drain_inst = self.nc.sync.drain()
        wait_clock.add_sem_waits(
            drain_inst.ins, ScopedClock({None: tick_clock.global_clock})
        )
        self.nc.all_engine_barrier(sem_only=True)
        popped = self.nc._tile_sem_poison_stack.pop()
        assert popped is self._sem_poison

    tile.TileContext._drain_and_barrier = _drain_and_barrier


def _build():
    """Build + compile the per-core Bass graph (cached)."""
    import concourse.bass as bass
    import concourse.tile as tile
    from concourse import bacc, mybir

    if os.environ.get("KERNEL_SLIM_TAIL", "1") == "1":
        _patch_tail_barrier(tile)

    bf16 = mybir.dt.bfloat16
    f32 = mybir.dt.float32
    AF = mybir.ActivationFunctionType

    nc = bacc.Bacc("TRN2", target_bir_lowering=False, debug=False,
                   num_devices=N_CORES)

    x_dram = nc.dram_tensor("x", [B_PER_CORE, DIM, N], bf16, kind="ExternalInput")
    wt_dram = nc.dram_tensor("wt", [DIM, 3 * DIM], bf16, kind="ExternalInput")
    embt_dram = nc.dram_tensor("embt", [D, N], bf16, kind="ExternalInput")
    out_dram = nc.dram_tensor("out", [B_PER_CORE, HEADS * D, N], bf16,
                              kind="ExternalOutput")

    with tile.TileContext(nc) as tc:
        with (
            tc.tile_pool(name="const", bufs=1) as const_pool,
            tc.tile_pool(name="xin", bufs=1) as x_pool,
            tc.tile_pool(name="qkv", bufs=1) as qkv_pool,
            tc.tile_pool(name="expsim", bufs=6) as exp_pool,
            tc.tile_pool(name="padd", bufs=4) as padd_pool,
            tc.tile_pool(name="rec", bufs=3) as rec_pool,
            tc.tile_pool(name="outsb", bufs=3) as out_pool,
            tc.tile_pool(name="wide_ps", bufs=2, space="PSUM") as wide_ps,
            tc.tile_pool(name="pv_ps", bufs=2, space="PSUM") as pv_ps,
        ):
            # ---- input DMAs, c-pipelined so group matmuls can chase chunk
            # arrivals. ----
            x_sb = [[[x_pool.tile([128, NH], bf16, tag=f"x{b}_{c}_{nh}",
                                  name=f"x{b}_{c}_{nh}")
                      for nh in range(2)] for c in range(CC)]
                    for b in range(B_PER_CORE)]
            wtv_sb = [const_pool.tile([128, DIM], bf16, tag=f"wtv{c}",
                                      name=f"wtv{c}") for c in range(CC)]
            wtq_sb = [const_pool.tile([128, DIM], bf16, tag=f"wtq{c}",
                                      name=f"wtq{c}") for c in range(CC)]
            wtk_sb = [const_pool.tile([128, DIM], bf16, tag=f"wtk{c}",
                                      name=f"wtk{c}") for c in range(CC)]
            embt_sb = const_pool.tile([D, N], bf16, tag="embt")

            for c in range(CC):
                nc.sync.dma_start(x_sb[0][c][0][:],
                                  x_dram[0, bass.ts(c, 128), 0:NH])
            for c in range(CC):
                nc.scalar.dma_start(wtv_sb[c][:],
                                    wt_dram[bass.ts(c, 128), 2 * DIM:3 * DIM])
            for c in range(CC):
                nc.gpsimd.dma_start(x_sb[0][c][1][:],
                                    x_dram[0, bass.ts(c, 128), NH:N])
            nc.gpsimd.dma_start(embt_sb[:], embt_dram[:])
            for c in range(CC):
                nc.sync.dma_start(wtk_sb[c][:],
                                  wt_dram[bass.ts(c, 128), DIM:2 * DIM])
            for c in range(CC):
                nc.scalar.dma_start(wtq_sb[c][:], wt_dram[bass.ts(c, 128), 0:DIM])
            for c in range(CC):
                for nh in range(2):
                    nc.sync.dma_start(x_sb[1][c][nh][:],
                                      x_dram[1, bass.ts(c, 128),
                                             bass.ts(nh, NH)])

            # ---- constants ----
            warm_sb = const_pool.tile([128, NH], bf16, tag="warm")
            nc.vector.memset(warm_sb[:], 1.0)
            ones128 = const_pool.tile([128, 128], bf16, tag="ones128")
            nc.vector.memset(ones128[:], 1.0)

            # preload the exp table set on ACT during the DMA wait
            exp_warm = const_pool.tile([1, 8], bf16, tag="exp_warm")
            nc.scalar.activation(exp_warm[:], warm_sb[0:1, 0:8], AF.Exp)

            # GpSimd ucode warm-up (first GpSimd tensor op pays ~6us IRAM
            # load; pay it in the DMA lead-in)
            gp_warm = const_pool.tile([128, 8], bf16, tag="gp_warm")
            nc.gpsimd.tensor_add(gp_warm[:], warm_sb[:, 0:8], warm_sb[:, 0:8])

            # ---- PE warm-up: junk matmuls while input DMAs are in flight;
            # flips the HAM clock gate toward 2.4 GHz before real work ----
            warm_ps = wide_ps.tile([128, 2 * NH], f32, tag="w", name="warm_ps")
            for i in range(10):
                nc.tensor.matmul(warm_ps[:, bass.ts(i % 2, NH)],
                                 warm_sb[:, 0:128], warm_sb[:],
                                 start=True, stop=True)
            warm_out = const_pool.tile([1, 8], f32, tag="warm_out")
            nc.vector.tensor_copy(warm_out[:], warm_ps[0:1, 0:8])
            warm_dram = nc.dram_tensor("warm_scratch", [1, 8], f32)
            nc.scalar.dma_start(warm_dram[:], warm_out[:])

            # ---- qkv staging ----
            q_sb = [qkv_pool.tile([128, HEADS * N], bf16, tag=f"q{b}",
                                  name=f"q{b}") for b in range(B_PER_CORE)]
            k_sb = [qkv_pool.tile([128, HEADS * N], bf16, tag=f"k{b}",
                                  name=f"k{b}") for b in range(B_PER_CORE)]
            v_sb = [qkv_pool.tile([128, NJ * DIM], bf16, tag=f"v{b}",
                                  name=f"v{b}") for b in range(B_PER_CORE)]

            # ---- projection emitters.  Whole-group versions for the
            # DMA-paced pre-attention phase; half-group closures (4 matmuls
            # each, second half appends the DVE consumer) for fillers. ----
            def emit_qk_group(b, h, which):
                first, second = make_qk_halves(b, h, which)
                first(); second()

            def emit_v_group(b, g):
                first, second = make_v_halves(b, g)
                first(); second()

            def make_qk_halves(b, h, which):
                st = {}
                wt_t = wtq_sb if which == "q" else wtk_sb

                def half(nh):
                    if nh == 0:
                        st["ps"] = wide_ps.tile([128, 2 * NH], f32, tag="w",
                                                name=f"{which}{b}_{h}")
                    ps = st["ps"]
                    for c in range(CC):
                        nc.tensor.matmul(
                            ps[:, bass.ts(nh, NH)],
                            wt_t[c][:, bass.ts(h, 128)],
                            x_sb[b][c][nh][:],
                            start=(c == 0), stop=(c == CC - 1),
                        )
                    if nh == 1:
                        if which == "q":
                            nc.vector.tensor_copy(q_sb[b][:, h * N:(h + 1) * N],
                                                  ps[:])
                        else:
                            nc.vector.tensor_add(k_sb[b][:, h * N:(h + 1) * N],
                                                 ps[:], embt_sb[:])

                return (lambda: half(0)), (lambda: half(1))

            def make_v_halves(b, g):
                st = {}

                def half(jo):
                    if jo == 0:
                        st["ps"] = wide_ps.tile([128, 2 * NH], f32, tag="w",
                                                name=f"v{b}_{g}")
                    ps = st["ps"]
                    j = 2 * g + jo
                    for c in range(CC):
                        nc.tensor.matmul(
                            ps[:, bass.ts(jo, NH)],
                            x_sb[b][c][j // 4][:, bass.ts(j % 4, 128)],
                            wtv_sb[c][:],
                            start=(c == 0), stop=(c == CC - 1),
                        )
                    if jo == 1:
                        nc.vector.tensor_copy(v_sb[b][:, bass.ts(g, 2 * NH)],
                                              ps[:])

                return (lambda: half(0)), (lambda: half(1))

            # ---- attention for one head ----
            # mode: "tree" = GpSimd/DVE padd tree + deferred ones-matmul;
            #       "direct" = denominators via interleaved accumulating
            #                  ones-matmuls (PE filler for starved heads);
            #       last head = direct + inline quarter-split normalize.
            def emit_attn_head(b, h, fillers, mode, last, deferred,
                              next_defer_at):
                q_h = q_sb[b][:, h * N:(h + 1) * N]
                k_h = k_sb[b][:, h * N:(h + 1) * N]
                pv = pv_ps.tile([128, 2 * NH], f32, tag="pv",
                                name=f"pv{b}_{h}")
                sums_w = None
                exs = [None] * NJ
                padds = {}
                defer_at = 0 if mode == "direct" else 3

                for jc in range(NJ):
                    if deferred is not None and jc == defer_at:
                        deferred()
                        deferred = None
                    sim = wide_ps.tile([128, 2 * NH], f32, tag="w",
                                       name=f"sim{b}_{h}_{jc}")
                    for ih in range(2):
                        nc.tensor.matmul(
                            sim[:, bass.ts(ih, NH)],
                            k_h[:, bass.ts(jc, 128)],
                            q_h[:, bass.ts(ih, NH)],
                            start=True, stop=True,
                        )
                    ex = exp_pool.tile([128, 2 * NH], bf16, tag="exp",
                                       name=f"ex{b}_{h}_{jc}")
                    if last and jc == NJ - 1:
                        # split final exp so the tail chain unblocks earlier
                        for ih in range(2):
                            nc.scalar.activation(ex[:, bass.ts(ih, NH)],
                                                 sim[:, bass.ts(ih, NH)],
                                                 AF.Exp)
                    else:
                        nc.scalar.activation(ex[:], sim[:], AF.Exp)
                    exs[jc] = ex
                    # pv lags one jc so its LDWEIGHTS prefetches during the
                    # sim stream
                    if jc > 0:
                        for ih in range(2):
                            nc.tensor.matmul(
                                pv[:, bass.ts(ih, NH)],
                                v_sb[b][:, (jc - 1) * NH + h * 128:
                                        (jc - 1) * NH + h * 128 + 128],
                                exs[jc - 1][:, bass.ts(ih, NH)],
                                start=(jc == 1), stop=False,
                            )
                    if mode == "direct":
                        if jc == 1:
                            pa = padd_pool.tile([128, 2 * NH], bf16,
                                                tag="padd",
                                                name=f"pa{b}_{h}_01")
                            nc.vector.tensor_add(pa[:], exs[0][:], exs[1][:])
                            padds["01"] = pa
                        elif jc == 2:
                            sums_w = pv_ps.tile([128, 2 * NH], f32, tag="pv",
                                                name=f"sums{b}_{h}")
                            for ih in range(2):
                                nc.tensor.matmul(
                                    sums_w[:, bass.ts(ih, NH)], ones128[:],
                                    padds["01"][:, bass.ts(ih, NH)],
                                    start=True, stop=False,
                                )
                        elif jc >= 3:
                            for ih in range(2):
                                nc.tensor.matmul(
                                    sums_w[:, bass.ts(ih, NH)], ones128[:],
                                    exs[jc - 1][:, bass.ts(ih, NH)],
                                    start=False, stop=False,
                                )
                    else:
                        if jc == 1:
                            pa = padd_pool.tile([128, 2 * NH], bf16,
                                                tag="padd_g0",
                                                name=f"pa{b}_{h}_01")
                            nc.gpsimd.tensor_add(pa[:], exs[0][:], exs[1][:])
                            padds["01"] = pa
                        elif jc == 3:
                            pa = padd_pool.tile([128, 2 * NH], bf16,
                                                tag="padd_g1",
                                                name=f"pa{b}_{h}_23")
                            nc.gpsimd.tensor_add(pa[:], exs[2][:], exs[3][:])
                            padds["23"] = pa
                        elif jc == 5:
                            pa = padd_pool.tile([128, 2 * NH], bf16,
                                                tag="padd_g2",
                                                name=f"pa2{b}_{h}_a")
                            nc.gpsimd.tensor_add(pa[:], padds["01"][:],
                                                 padds["23"][:])
                            padds["0123"] = pa
                            pa = padd_pool.tile([128, 2 * NH], bf16,
                                                tag="padd",
                                                name=f"pa{b}_{h}_45")
                            nc.vector.tensor_add(pa[:], exs[4][:], exs[5][:])
                            padds["45"] = pa
                        elif jc == 7:
                            pa = padd_pool.tile([128, 2 * NH], bf16,
                                                tag="padd2",
                                                name=f"pa{b}_{h}_67")
                            nc.vector.tensor_add(pa[:], exs[6][:], exs[7][:])
                            padds["67"] = pa
                    for f in fillers.get(jc, ()):
                        f()

                if not last:
                    # final pv pair (j-chunk NJ-1)
                    for ih in range(2):
                        nc.tensor.matmul(
                            pv[:, bass.ts(ih, NH)],
                            v_sb[b][:, (NJ - 1) * NH + h * 128:
                                    (NJ - 1) * NH + h * 128 + 128],
                            exs[NJ - 1][:, bass.ts(ih, NH)],
                            start=False, stop=True,
                        )
                if mode == "direct":
                    for ih in range(2):
                        nc.tensor.matmul(
                            sums_w[:, bass.ts(ih, NH)], ones128[:],
                            exs[NJ - 1][:, bass.ts(ih, NH)],
                            start=False, stop=True,
                        )
                        if last:
                            nc.tensor.matmul(
                                pv[:, bass.ts(ih, NH)],
                                v_sb[b][:, (NJ - 1) * NH + h * 128:
                                        (NJ - 1) * NH + h * 128 + 128],
                                exs[NJ - 1][:, bass.ts(ih, NH)],
                                start=False, stop=True,
                            )
                    if last:
                        # inline quarter-split normalize: recip/mul/DMA in
                        # 256-wide quarters pipelined across sync/scalar
                        rec = rec_pool.tile([128, 2 * NH], f32, tag="rec",
                                            name=f"rec{b}_{h}")
                        o = out_pool.tile([128, 2 * NH], bf16, tag="o",
                                          name=f"o{b}_{h}")
                        NQ = NH // 2
                        for iq in range(4):
                            nc.vector.reciprocal_approx_fast(
                                out=rec[:, bass.ts(iq, NQ)],
                                in_=sums_w[:, bass.ts(iq, NQ)])
                            nc.vector.tensor_mul(o[:, bass.ts(iq, NQ)],
                                                 pv[:, bass.ts(iq, NQ)],
                                                 rec[:, bass.ts(iq, NQ)])
                            eng = nc.sync if iq % 2 == 0 else nc.scalar
                            eng.dma_start(
                                out_dram[b, h * D:(h + 1) * D,
                                         bass.ts(iq, NQ)],
                                o[:, bass.ts(iq, NQ)])
                        return None

                    def finish_direct():
                        rec = rec_pool.tile([128, 2 * NH], f32, tag="rec",
                                            name=f"rec{b}_{h}")
                        nc.vector.reciprocal_approx_fast(out=rec[:],
                                                         in_=sums_w[:])
                        o = out_pool.tile([128, 2 * NH], bf16, tag="o",
                                          name=f"o{b}_{h}")
                        nc.vector.tensor_mul(o[:], pv[:], rec[:])
                        nc.sync.dma_start(out_dram[b, h * D:(h + 1) * D, :],
                                          o[:])

                    return finish_direct

                # tree heads: late tree levels on DVE
                pa2_1 = padd_pool.tile([128, 2 * NH], bf16, tag="padd2",
                                       name=f"pa2{b}_{h}_b")
                nc.vector.tensor_add(pa2_1[:], padds["45"][:], padds["67"][:])
                pa3 = padd_pool.tile([128, 2 * NH], bf16, tag="padd3",
                                     name=f"pa3{b}_{h}")
                nc.vector.tensor_add(pa3[:], padds["0123"][:], pa2_1[:])

                def finish_tree():
                    sums = wide_ps.tile([128, 2 * NH], f32, tag="w",
                                        name=f"sums{b}_{h}")
                    for ih in range(2):
                        nc.tensor.matmul(
                            sums[:, bass.ts(ih, NH)], ones128[:],
                            pa3[:, bass.ts(ih, NH)],
                            start=True, stop=True,
                        )
                    rec = rec_pool.tile([128, 2 * NH], f32, tag="rec",
                                        name=f"rec{b}_{h}")
                    nc.vector.reciprocal_approx_fast(out=rec[:], in_=sums[:])
                    o = out_pool.tile([128, 2 * NH], bf16, tag="o",
                                      name=f"o{b}_{h}")
                    nc.vector.tensor_mul(o[:], pv[:], rec[:])
                    nc.sync.dma_start(out_dram[b, h * D:(h + 1) * D, :], o[:])

                return finish_tree

            # ---- program order ----
            # pre-attention: batch-0 prerequisites, DMA-paced
            emit_v_group(0, 0)
            emit_v_group(0, 1)
            emit_v_group(0, 2)
            emit_v_group(0, 3)
            emit_qk_group(0, 0, "k")
            emit_qk_group(0, 0, "q")

            # half-group filler schedules: next head's k/q early in each
            # head (their DVE consumers finish before that head starts),
            # b1 v groups later
            def sched(*pairs):
                """pairs of (jc, closure) -> {jc: [closures]}"""
                d = {}
                for jc, f in pairs:
                    d.setdefault(jc, []).append(f)
                return d

            def halves(kind, *args):
                if kind == "qk":
                    return make_qk_halves(*args)
                return make_v_halves(*args)

            FILL = {}
            k01 = make_qk_halves(0, 1, "k"); q01 = make_qk_halves(0, 1, "q")
            FILL[(0, 0)] = sched((0, k01[0]), (1, k01[1]),
                                 (4, q01[0]), (5, q01[1]))
            k02 = make_qk_halves(0, 2, "k"); q02 = make_qk_halves(0, 2, "q")
            v10 = make_v_halves(1, 0)
            FILL[(0, 1)] = sched((0, k02[0]), (1, k02[1]),
                                 (2, q02[0]), (4, q02[1]),
                                 (5, v10[0]), (6, v10[1]))
            k03 = make_qk_halves(0, 3, "k"); q03 = make_qk_halves(0, 3, "q")
            v11 = make_v_halves(1, 1)
            FILL[(0, 2)] = sched((0, k03[0]), (1, k03[1]),
                                 (2, q03[0]), (4, q03[1]),
                                 (5, v11[0]), (6, v11[1]))
            k10 = make_qk_halves(1, 0, "k"); q10 = make_qk_halves(1, 0, "q")
            v12 = make_v_halves(1, 2); v13 = make_v_halves(1, 3)
            FILL[(0, 3)] = sched((0, k10[0]), (1, k10[1]),
                                 (2, q10[0]), (3, q10[1]),
                                 (4, v12[0]), (5, v12[1]),
                                 (6, v13[0]), (7, v13[1]))
            k11 = make_qk_halves(1, 1, "k"); q11 = make_qk_halves(1, 1, "q")
            FILL[(1, 0)] = sched((0, k11[0]), (1, k11[1]),
                                 (4, q11[0]), (5, q11[1]))
            k12 = make_qk_halves(1, 2, "k"); q12 = make_qk_halves(1, 2, "q")
            FILL[(1, 1)] = sched((1, k12[0]), (3, k12[1]),
                                 (5, q12[0]), (7, q12[1]))
            k13 = make_qk_halves(1, 3, "k"); q13 = make_qk_halves(1, 3, "q")
            FILL[(1, 2)] = sched((1, k13[0]), (3, k13[1]),
                                 (5, q13[0]), (7, q13[1]))
            FILL[(1, 3)] = {}

            MODE = {
                (0, 0): "tree", (0, 1): "tree", (0, 2): "tree",
                (0, 3): "tree",
                (1, 0): "tree", (1, 1): "direct", (1, 2): "direct",
                (1, 3): "direct",
            }

            deferred = None
            for b in range(B_PER_CORE):
                for h in range(HEADS):
                    last = (b == B_PER_CORE - 1 and h == HEADS - 1)
                    deferred = emit_attn_head(
                        b, h, FILL[(b, h)], MODE[(b, h)], last, deferred,
                        next_defer_at=None)

    nc.compile()
    return nc


def _get_compiled():
    if "nc" not in _COMPILED:
        _COMPILED["nc"] = _build()
    return _COMPILED["nc"]


def _run(fmap, w_qkv, emb_h, emb_w, **spmd_kwargs):
    from concourse.bass_utils import run_bass_kernel_spmd

    nc = _get_compiled()

    fmap = np.asarray(fmap, dtype=np.float32)
    w_qkv = np.asarray(w_qkv, dtype=np.float32)
    emb_h = np.asarray(emb_h, dtype=np.float32)
    emb_w = np.asarray(emb_w, dtype=np.float32)

    b, c, hh, ww = fmap.shape
    x = fmap.reshape(b, c, hh * ww)

    # fold q scale into weight rows, transpose to [c, o], cast to bf16
    w = w_qkv.copy()
    w[:HEADS * D] *= SCALE
    wt = np.ascontiguousarray(w.T).astype(_BF16)

    embt = np.ascontiguousarray(
        (emb_h[:, None, :] + emb_w[None, :, :]).reshape(N, D).T
    ).astype(_BF16)

    x16 = x.astype(_BF16)
    in_maps = [
        {
            "x": np.ascontiguousarray(x16[i * B_PER_CORE:(i + 1) * B_PER_CORE]),
            "wt": wt,
            "embt": embt,
        }
        for i in range(N_CORES)
    ]

    res = run_bass_kernel_spmd(nc, in_maps, core_ids=list(range(N_CORES)),
                               **spmd_kwargs)
    out = np.concatenate(
        [np.asarray(res.results[i]["out"], dtype=np.float32)
         for i in range(N_CORES)], axis=0)
    return out.reshape(B, HEADS * D, hh, ww), res


def kernel(fmap, w_qkv, emb_h, emb_w):
    out, _ = _run(fmap, w_qkv, emb_h, emb_w)
    return out


if __name__ == "__main__":
    rng = np.random.default_rng(0)
    fmap = rng.standard_normal((B, DIM, 32, 32), dtype=np.float32)
    w_qkv = rng.standard_normal((3 * HEADS * D, DIM), dtype=np.float32) * DIM ** -0.5
    emb_h = rng.standard_normal((32, D), dtype=np.float32) * SCALE
    emb_w = rng.standard_normal((32, D), dtype=np.float32) * SCALE
    out = kernel(fmap=fmap, w_qkv=w_qkv, emb_h=emb_h, emb_w=emb_w)
    print("kernel out:", out.shape, out.dtype)
